# revision 26
# baseline (speedup 1.0000x reference)
"""Trainium2 Bass kernel for BinaryLinearUnit:
    y = sign(x) @ sign(w).T ; BatchNorm1d(train) ; * gamma + beta

Strategy: data-parallel over the batch dim across 8 NeuronCores.
Each core computes y.T for its 1024-row batch slice with an FP8
(DoubleRow) matmul on the tensor engine — sign values are exactly
representable in fp8e4m3, and PSUM accumulates in fp32, so the matmul
is exact. Per-core BN partial stats (mean, E[y^2] per channel) are
combined with an 8-core AllReduce, then each core normalizes its slice
and writes y.T back.

The output features are processed in NSPLIT groups; each group's
AllReduce is issued as soon as its matmuls finish, and its coefficient
math + normalization are emitted one group later, so all BN work except
the last group's overlaps the remaining matmuls. Engine assignment is
chosen so no in-order engine queue blocks on collective latency:
  PE: matmuls | ACT: sign + sqrt + output-store DMA triggers |
  DVE: bn_stats, y-cast, stats math | GpSimd: collectives, readback
  DMA, normalize mul-add | Sync: input DMA.

Host side only reshapes/transposes: x and w are fed K-major (the
contraction dim IN must sit on SBUF partitions for the PE), the
per-core output arrives as y.T and is transposed back.
"""

import numpy as np

import concourse.bass as bass
import concourse.mybir as mybir
import concourse.tile as tile
from concourse import bacc
from concourse.bass import ts
from concourse.bass_utils import run_bass_kernel_spmd

N_CORES = 8
BN_EPS = 1e-5

f32 = mybir.dt.float32
f16 = mybir.dt.float16
fp8 = mybir.dt.float8e4


def build(B, IN, OUT, n_cores=N_CORES, use_fp8=True):
    """Build the per-core SPMD module. Shapes: x [B, IN], w [OUT, IN]."""
    Bc = B // n_cores          # batch rows per core
    KT = IN // 128             # k tiles (contraction)
    OT = OUT // 128            # output-feature tiles
    NB = min(512, Bc)          # matmul free dim / psum bank width
    BT = Bc // NB              # b tiles per core
    # BN stats groups over the output tiles: first group's AllReduce fires
    # early (absorbs inter-core skew), last group is tiny (short tail).
    if OT >= 16:
        GS = [OT // 4, (3 * OT // 4 - 2 + 1) // 2, 0, 2]
        GS[2] = OT - GS[0] - GS[1] - GS[3]
    else:
        GS = [OT - OT // 2, OT // 2]
    NSPLIT = len(GS)
    GO = [sum(GS[:q]) for q in range(NSPLIT)]   # group start offsets

    act_dt = fp8 if use_fp8 else mybir.dt.bfloat16

    nc = bacc.Bacc("TRN2", target_bir_lowering=False, debug=False,
                   num_devices=n_cores)

    # Per-core external I/O (host pre-transposed, K-major):
    #   xt[k, b] = x[core*Bc + b, k]
    #   w2[ot, p, ks, o] = w[ot*128 + o, ks*128 + p]
    #   yt[o, b] = out[core*Bc + b, o]
    xt = nc.dram_tensor("xt", [IN, Bc], f32, kind="ExternalInput")
    w2 = nc.dram_tensor("w2", [OT, 128, KT, 128], f32, kind="ExternalInput")
    # gb[p, 0, t] = gamma[t*128+p], gb[p, 1, t] = beta[t*128+p] (host-packed
    # so the load is one small contiguous DMA, not a 4-byte-strided gather)
    gb = nc.dram_tensor("gb", [128, 2, OT], f32, kind="ExternalInput")
    yt = nc.dram_tensor("yt", [OUT, Bc], f32, kind="ExternalOutput")

    # Collective bounce buffers per group: [mean/8, E[y^2]/8] per channel
    ccin = [
        nc.dram_tensor(f"ccin{q}", [128, 2 * GS[q]], f32) for q in range(NSPLIT)
    ]
    ccout = [
        nc.dram_tensor(f"ccout{q}", [128, 2 * GS[q]], f32, addr_space="Shared")
        for q in range(NSPLIT)
    ]

    with tile.TileContext(nc) as tc:
        with (
            tc.tile_pool(name="big", bufs=1) as big,
            tc.tile_pool(name="xs", bufs=3) as xsp,
            tc.tile_pool(name="ws", bufs=6) as wsp,
            tc.tile_pool(name="sw", bufs=3) as swp,
            tc.tile_pool(name="ps", bufs=2, space="PSUM") as psp,
            tc.tile_pool(name="st", bufs=2) as stp,
            tc.tile_pool(name="outp", bufs=4) as outp,
        ):
            # Standing tensors
            sxT = big.tile([128, KT, Bc], act_dt)       # sign(x).T, K-major
            yTt = big.tile([128, OT, Bc], f16)          # y.T (exact in fp16)
            mvT = big.tile([128, 2, OT], f32)           # per-core [mean, var]
            gbt = big.tile([128, 2, OT], f32)           # [gamma; beta]
            scal = big.tile([128, OT], f32)             # gamma * rstd
            nbias = big.tile([128, OT], f32)            # beta - mean * scal
            grTs = [None] * NSPLIT                      # global stats tiles

            def w_chain(ot):
                # two half-K chunks for finer DMA/ACT pipelining
                swt = swp.tile([128, KT, 128], act_dt, tag="swt", name="swt")
                hk = KT // 2
                for h in range(2):
                    wst = wsp.tile([128, hk, 128], f32, tag="wst", name="wst")
                    nc.sync.dma_start(
                        out=wst[:], in_=w2[ot, :, h * hk : (h + 1) * hk, :]
                    )
                    nc.scalar.sign(swt[:, h * hk : (h + 1) * hk, :], wst[:])
                return swt

            def mm_tile(ot, swt, bt_outer=False):
                psums = [
                    psp.tile([128, NB], f32, tag=f"ps{bt}", name=f"psum{bt}")
                    for bt in range(BT)
                ]
                if use_fp8:
                    KP = KT // 2
                    # bt-outer consumes the x halves progressively (startup);
                    # kp-outer reuses each weight load across b tiles (steady).
                    order = (
                        [(kp, bt) for bt in range(BT) for kp in range(KP)]
                        if bt_outer
                        else [(kp, bt) for kp in range(KP) for bt in range(BT)]
                    )
                    for kp, bt in order:
                        nc.tensor.matmul(
                            psums[bt][:],
                            lhsT=swt[:, 2 * kp : 2 * kp + 2, :],
                            rhs=sxT[:, 2 * kp : 2 * kp + 2, ts(bt, NB)],
                            start=(kp == 0),
                            stop=(kp == KP - 1),
                            perf_mode=mybir.MatmulPerfMode.DoubleRow,
                        )
                else:
                    for k in range(KT):
                        for bt in range(BT):
                            nc.tensor.matmul(
                                psums[bt][:],
                                lhsT=swt[:, k, :],
                                rhs=sxT[:, k, ts(bt, NB)],
                                start=(k == 0),
                                stop=(k == KT - 1),
                            )
                st6 = stp.tile([128, BT, 6], f32, tag="st6", name="st6", bufs=4)
                for bt in range(BT):
                    nc.vector.bn_stats(st6[:, bt, :], psums[bt][:])
                    nc.vector.tensor_copy(yTt[:, ot, ts(bt, NB)], psums[bt][:])
                nc.vector.bn_aggr(mvT[:, :, ot], st6[:])

            def stats_pre(q):
                """Per-core partial stats -> AllReduce, right after group q's
                matmuls. DVE arith + Sync DMA + GpSimd collective/readback."""
                o0, HOT = GO[q], GS[q]
                osl = slice(o0, o0 + HOT)
                arT = stp.tile([128, 2, HOT], f32, tag="arT", name="arT")
                tmp = stp.tile([128, HOT], f32, tag="tmp_ar", name="tmp_ar")
                nc.vector.tensor_scalar_mul(arT[:, 0, :], mvT[:, 0, osl], 1.0 / n_cores)
                nc.vector.tensor_mul(tmp[:], mvT[:, 0, osl], mvT[:, 0, osl])
                nc.vector.tensor_add(tmp[:], tmp[:], mvT[:, 1, osl])
                nc.vector.tensor_scalar_mul(arT[:, 1, :], tmp[:], 1.0 / n_cores)
                nc.sync.dma_start(out=ccin[q][:], in_=arT[:])
                nc.gpsimd.collective_compute(
                    "AllReduce",
                    mybir.AluOpType.add,
                    replica_groups=[list(range(n_cores))],
                    ins=[ccin[q][:]],
                    outs=[ccout[q][:]],
                )
                grT = big.tile([128, 2, HOT], f32, name=f"grT{q}")  # noqa
                # SWDGE readback keeps the Sync HWDGE queue free for the
                # next group's weight prefetch (queues are in-order).
                nc.gpsimd.dma_start(out=grT[:], in_=ccout[q][:])
                grTs[q] = grT

            def stats_post(q):
                """Global stats -> scale/bias for group q (emitted one group
                later, when the AllReduce has long finished)."""
                o0, HOT = GO[q], GS[q]
                osl = slice(o0, o0 + HOT)
                grT = grTs[q]
                gmean = grT[:, 0, :]
                gvar = stp.tile([128, HOT], f32, tag="gvar", name="gvar")
                veps = stp.tile([128, HOT], f32, tag="veps", name="veps")
                nc.vector.tensor_mul(gvar[:], gmean, gmean)
                nc.vector.tensor_sub(gvar[:], grT[:, 1, :], gvar[:])
                nc.vector.tensor_scalar_add(veps[:], gvar[:], BN_EPS)
                sq = stp.tile([128, HOT], f32, tag="sq", name="sq")
                nc.scalar.sqrt(sq[:], veps[:])
                r = stp.tile([128, HOT], f32, tag="r", name="rstd")
                nc.vector.reciprocal(r[:], sq[:])
                t2 = stp.tile([128, HOT], f32, tag="t2", name="t2")
                for _ in range(2):  # Newton: r <- r * (1.5 - 0.5 * veps * r^2)
                    nc.vector.tensor_mul(t2[:], veps[:], r[:])
                    nc.vector.tensor_mul(t2[:], t2[:], r[:])
                    nc.vector.tensor_scalar(t2[:], t2[:], -0.5, 1.5,
                                            op0=mybir.AluOpType.mult,
                                            op1=mybir.AluOpType.add)
                    nc.vector.tensor_mul(r[:], r[:], t2[:])
                nc.vector.tensor_mul(scal[:, osl], gbt[:, 0, osl], r[:])
                nc.vector.tensor_mul(t2[:], gmean, scal[:, osl])
                nc.vector.tensor_sub(nbias[:, osl], gbt[:, 1, osl], t2[:])

            def norm_group(q, last=False):
                # DVE mul-add (fp16 src -> 2x mode, and the DVE queue reaches
                # this position right at the group boundary, when the
                # coefficients are already in hand). Mid-phase stores ride
                # the lightly-loaded Sync HWDGE; the last group's stores use
                # the by-then-idle ACT HWDGE so the tail pipelines across
                # two engines.
                for ot in range(GO[q], GO[q] + GS[q]):
                    ob = outp.tile([128, Bc], f32, tag="ob", name="ob")
                    nc.vector.tensor_scalar(
                        ob[:],
                        yTt[:, ot, :],
                        scal[:, ot : ot + 1],
                        nbias[:, ot : ot + 1],
                        op0=mybir.AluOpType.mult,
                        op1=mybir.AluOpType.add,
                    )
                    eng = nc.scalar if last else nc.sync
                    eng.dma_start(out=yt[ts(ot, 128), :], in_=ob[:])

            # ---- emission order == scheduling priority ----
            # ot=0 weight chain first so the PE can start ASAP
            swt_next = w_chain(0)

            # x sign, one full-width chunk per k tile (progressively consumed
            # by the bt-outer matmul order of the first output tiles)
            for ks in range(KT):
                xst = xsp.tile([128, Bc], f32, tag="xst", name="xst")
                nc.sync.dma_start(out=xst[:], in_=xt[ts(ks, 128), :])
                nc.scalar.sign(sxT[:, ks, :], xst[:])

            nc.sync.dma_start(out=gbt[:], in_=gb[:])

            for q in range(NSPLIT):
                for ot in range(GO[q], GO[q] + GS[q]):
                    swt = swt_next
                    if ot + 1 < OT:
                        swt_next = w_chain(ot + 1)
                    mm_tile(ot, swt, bt_outer=(ot < 2))
                # trigger this group's AllReduce the moment its stats exist
                stats_pre(q)
                if q >= 1:
                    stats_post(q - 1)
                    norm_group(q - 1)
            stats_post(NSPLIT - 1)
            norm_group(NSPLIT - 1, last=True)

    nc.finalize()
    return nc


def shard_inputs(x, w, gamma, beta, n_cores=N_CORES):
    B, IN = x.shape
    OUT = w.shape[0]
    Bc = B // n_cores
    KT, OT = IN // 128, OUT // 128
    w2 = np.ascontiguousarray(
        w.reshape(OT, 128, KT, 128).transpose(0, 3, 2, 1)
    )
    gbp = np.ascontiguousarray(
        np.stack([gamma.reshape(OT, 128).T, beta.reshape(OT, 128).T], axis=1)
    )
    in_maps = []
    for c in range(n_cores):
        xt = np.ascontiguousarray(x[c * Bc : (c + 1) * Bc].T)
        in_maps.append({"xt": xt, "w2": w2, "gb": gbp})
    return in_maps


_NC_CACHE = {}


def kernel(x, w, gamma, beta):
    x = np.asarray(x)
    w = np.asarray(w)
    gamma = np.asarray(gamma)
    beta = np.asarray(beta)
    B, IN = x.shape
    OUT = w.shape[0]

    key = (B, IN, OUT)
    if key not in _NC_CACHE:
        _NC_CACHE[key] = build(B, IN, OUT)
    nc = _NC_CACHE[key]

    in_maps = shard_inputs(x, w, gamma, beta)
    res = run_bass_kernel_spmd(nc, in_maps, list(range(N_CORES)))
    out = np.concatenate([r["yt"] for r in res.results], axis=1).T
    return np.ascontiguousarray(out)


if __name__ == "__main__":
    rng = np.random.default_rng(0)
    B, IN, OUT = 8192, 4096, 4096
    x = rng.standard_normal((B, IN)).astype(np.float32)
    w = rng.standard_normal((OUT, IN)).astype(np.float32)
    gamma = np.ones(OUT, np.float32)
    beta = np.zeros(OUT, np.float32)
    out = kernel(x, w, gamma, beta)
    print(out.shape, out.dtype)
